# revision 1
# baseline (speedup 1.0000x reference)
"""Causal self-attention (RMSNorm-QK + RoPE) Trainium2 Bass kernel.

Problem: B=2, T=2048, C=1024, H=16 heads, D=64.
Sharding: 8 cores = 2 (batch) x 4 (head groups of 4 heads).
Each core computes q/k/v projections for its 4 heads, attention, and a
partial output projection (column-parallel over heads); the host sums the
4 partials per batch and transposes.

All matmuls run in float32r (TF32-like, ~13-bit mantissa, 4x fp32 matmul
speed). f32r matmul operands must be produced by rounding ops or f32r DMA;
host pre-rounds the DRAM inputs.

Per-core layouts ("T-layout" = channels on partitions, tokens free):
  projection chunks [128, 512]: row 32h+i = head h, rope-half dim i
  qT_r/kT_r  2 x [128, 2048] f32r : chunk c rows 64*(h%2)+d = head 2c+h%2
  v_r        16 x [128, 260] f32r : head h at cols 65h..65h+63, ones col
  scoresT    [s-chunk 128, t-block 512]; softmax denom = ones-column row
  yT_sb      2 x [128, 2048] f32r : pair chunk c = heads (2c, 2c+1)
Output: outT [1024, 2048] = (partial out).T per core; host sums + transposes.
"""

import sys

for _p in ("/opt/trn_rl_repo",):
    if _p not in sys.path:
        sys.path.append(_p)

import numpy as np

B, T, C = 2, 2048, 1024
H_TOT, D = 16, 64
HPC = 4               # heads per core
N_CORES = 8
P = 128               # partitions
NB = 4                # t-blocks of 512
TB = 512              # t-block size
KCH = 8               # C / 128 contraction chunks
VW = 65 * HPC         # v width with ones columns = 260
RMS_EPS = 1.1920928955078125e-07
ROPE_BASE = 10000.0

_CACHE = {}


def _build_consts():
    """Host-side constant tensors shared by all cores."""
    inv_freq = (1.0 / (ROPE_BASE ** (np.arange(0, D, 2, dtype=np.float32) / np.float32(D)))).astype(np.float32)
    pos = np.arange(T, dtype=np.float32)
    freqs = np.outer(pos, inv_freq).astype(np.float32)      # [T, 32]
    cos = np.cos(freqs).astype(np.float32)                  # [T, 32]
    sin = np.sin(freqs).astype(np.float32)
    cosr = np.ascontiguousarray(np.tile(cos.T, (HPC, 1)))   # [128, T]
    sinr = np.ascontiguousarray(np.tile(sin.T, (HPC, 1)))
    # ind32 [128, 4]: per-32-row-group summing matrix (lhsT for RMS sums)
    ind32 = np.zeros((P, HPC), dtype=np.float32)
    for p_ in range(P):
        ind32[p_, p_ // 32] = 1.0
    # bc32 [4, 128]: broadcast inv (4 heads) to 32-row groups (lhsT)
    bc32 = np.zeros((HPC, P), dtype=np.float32)
    for p_ in range(P):
        bc32[p_ // 32, p_] = 1.0
    # selpair [128, 256]: chunk c (=0,1): col m -> den row 32*(2c + m//64)
    selpair = np.zeros((P, 2 * P), dtype=np.float32)
    for c in range(2):
        for m in range(P):
            selpair[32 * (2 * c + m // 64), 128 * c + m] = 1.0
    return dict(cosr=cosr, sinr=sinr, ind32=ind32, bc32=bc32,
                selpair=selpair)


def _build_module():
    import concourse.bacc as bacc
    import concourse.mybir as mybir
    import concourse.tile as tile

    f32 = mybir.dt.float32
    f32r = mybir.dt.float32r
    Exp = mybir.ActivationFunctionType.Exp
    Ln = mybir.ActivationFunctionType.Ln
    Alu = mybir.AluOpType

    nc = bacc.Bacc("TRN2", target_bir_lowering=False, debug=False,
                   num_devices=N_CORES)

    xt_d = nc.dram_tensor("xt", [C, T], f32r, kind="ExternalInput").ap()
    wq_d = nc.dram_tensor("wq", [C, 256], f32r, kind="ExternalInput").ap()
    wk_d = nc.dram_tensor("wk", [C, 256], f32r, kind="ExternalInput").ap()
    wv_d = nc.dram_tensor("wv", [C, VW], f32r, kind="ExternalInput").ap()
    wp_d = nc.dram_tensor("wp", [256, C], f32r, kind="ExternalInput").ap()
    cosr_d = nc.dram_tensor("cosr", [P, T], f32, kind="ExternalInput").ap()
    sinr_d = nc.dram_tensor("sinr", [P, T], f32, kind="ExternalInput").ap()
    ind32_d = nc.dram_tensor("ind32", [P, HPC], f32r, kind="ExternalInput").ap()
    bc32_d = nc.dram_tensor("bc32", [HPC, P], f32r, kind="ExternalInput").ap()
    selpair_d = nc.dram_tensor("selpair", [P, 2 * P], f32r, kind="ExternalInput").ap()
    zeros_d = nc.dram_tensor("zeros", [64, T], f32r, kind="ExternalInput").ap()
    out_d = nc.dram_tensor("outT", [C, T], f32, kind="ExternalOutput").ap()

    with tile.TileContext(nc) as tc:
        with (
            tc.tile_pool(name="sb", bufs=1) as sb,
            tc.tile_pool(name="trans", bufs=2) as tr,
            tc.tile_pool(name="ps", bufs=2, space="PSUM") as ps,
        ):
            # ---- constants / weights in (direct f32r DMA) ----
            def direct_load(name, dram_slice, shape, dt=f32r):
                t_r = sb.tile(shape, dt, tag=name, name=name)
                nc.sync.dma_start(out=t_r[:], in_=dram_slice)
                return t_r

            ind32_r = direct_load("ind32r", ind32_d[:, :], [P, HPC])
            bc32_r = direct_load("bc32r", bc32_d[:, :], [HPC, P])
            selpair_r = direct_load("selpairr", selpair_d[:, :], [P, 2 * P])
            cosr_t = direct_load("cosr", cosr_d[:, :], [P, T], f32)
            sinr_t = direct_load("sinr", sinr_d[:, :], [P, T], f32)
            wq_r = [direct_load(f"wqr{k}", wq_d[k * P:(k + 1) * P, :], [P, 256])
                    for k in range(KCH)]
            wk_r = [direct_load(f"wkr{k}", wk_d[k * P:(k + 1) * P, :], [P, 256])
                    for k in range(KCH)]
            wv_r = [direct_load(f"wvr{k}", wv_d[k * P:(k + 1) * P, :], [P, VW])
                    for k in range(KCH)]
            wp_r = [direct_load(f"wpr{c}", wp_d[c * P:(c + 1) * P, :], [P, C])
                    for c in range(2)]

            # ---- persistent intermediates ----
            qT_r = [sb.tile([P, T], f32r, tag=f"qT{c}", name=f"qT{c}")
                    for c in range(2)]
            # kTe[c]: rows 0-63 = head 2c, rows 64-127 zero;
            # kTo[c]: rows 0-63 zero, rows 64-127 = head 2c+1.
            kTe = [sb.tile([P, T], f32r, tag=f"kTe{c}", name=f"kTe{c}")
                   for c in range(2)]
            kTo = [sb.tile([P, T], f32r, tag=f"kTo{c}", name=f"kTo{c}")
                   for c in range(2)]
            for c in range(2):
                nc.sync.dma_start(out=kTe[c][64:128, :], in_=zeros_d[:, :])
                nc.sync.dma_start(out=kTo[c][0:64, :], in_=zeros_d[:, :])
            v_r = [sb.tile([P, VW], f32r, tag=f"v{s}", name=f"v{s}")
                   for s in range(T // P)]
            yT_sb = [sb.tile([P, T], f32r, tag=f"yT{c}", name=f"yT{c}")
                     for c in range(2)]
            den_stack = sb.tile([P, T], f32r, tag="denstack", name="den_stack")
            nc.gpsimd.memset(den_stack[:].bitcast(f32), 1.0)
            eps_t = sb.tile([HPC, 1], f32, tag="epst", name="eps_t")
            nc.gpsimd.memset(eps_t[:], RMS_EPS)

            # ====== Phase 1+2: projections, RMS-norm, RoPE, repack ======
            for n in range(NB):
                nsl = slice(n * TB, (n + 1) * TB)
                xr_t = []
                for k in range(KCH):
                    xr = tr.tile([P, TB], f32r, tag="xr", name=f"xr{n}_{k}", bufs=9)
                    nc.sync.dma_start(out=xr[:], in_=xt_d[k * P:(k + 1) * P, nsl])
                    xr_t.append(xr)
                pq0 = ps.tile([P, TB], f32, tag="psA", name=f"pq0_{n}")
                pq1 = ps.tile([P, TB], f32, tag="psA", name=f"pq1_{n}")
                pk0 = ps.tile([P, TB], f32, tag="psB", name=f"pk0_{n}")
                pk1 = ps.tile([P, TB], f32, tag="psB", name=f"pk1_{n}")
                for k in range(KCH):
                    xr = xr_t[k]
                    st = (k == 0)
                    sp = (k == KCH - 1)
                    nc.tensor.matmul(pq0[:], lhsT=wq_r[k][:, 0:128], rhs=xr[:],
                                     start=st, stop=sp)
                    nc.tensor.matmul(pq1[:], lhsT=wq_r[k][:, 128:256], rhs=xr[:],
                                     start=st, stop=sp)
                    nc.tensor.matmul(pk0[:], lhsT=wk_r[k][:, 0:128], rhs=xr[:],
                                     start=st, stop=sp)
                    nc.tensor.matmul(pk1[:], lhsT=wk_r[k][:, 128:256], rhs=xr[:],
                                     start=st, stop=sp)
                # q/k chunks out of PSUM
                x1q = tr.tile([P, TB], f32, tag="x1q", name=f"x1q{n}", bufs=1)
                x2q = tr.tile([P, TB], f32, tag="x2q", name=f"x2q{n}", bufs=1)
                x1k = tr.tile([P, TB], f32, tag="x1k", name=f"x1k{n}", bufs=1)
                x2k = tr.tile([P, TB], f32, tag="x2k", name=f"x2k{n}", bufs=1)
                nc.vector.tensor_copy(x1q[:], pq0[:])
                nc.vector.tensor_copy(x2q[:], pq1[:])
                nc.vector.tensor_copy(x1k[:], pk0[:])
                nc.vector.tensor_copy(x2k[:], pk1[:])
                # v projections (second sub-pass over the same xr tiles)
                pv = [ps.tile([P, VW], f32, tag=("psA" if s < 2 else "psB"),
                              name=f"pv{n}_{s}") for s in range(4)]
                for k in range(KCH):
                    st = (k == 0)
                    sp = (k == KCH - 1)
                    for s_rel in range(4):
                        nc.tensor.matmul(
                            pv[s_rel][:],
                            lhsT=xr_t[k][:, s_rel * P:(s_rel + 1) * P],
                            rhs=wv_r[k][:], start=st, stop=sp)
                for s_rel in range(4):
                    vt = v_r[4 * n + s_rel]
                    nc.vector.tensor_copy(vt[:], pv[s_rel][:])
                    nc.vector.tensor_scalar(vt[:, 64:VW:65], pv[s_rel][:, 64:VW:65],
                                            0.0, 1.0, Alu.mult, Alu.add)
                # RMS-norm + RoPE + repack, per tensor
                for (x1, x2, dstT, eng) in ((x1q, x2q, qT_r, "q"),
                                            (x1k, x2k, None, "k")):
                    e = nc.vector if eng == "q" else nc.gpsimd
                    sq1 = tr.tile([P, TB], f32r, tag="tmpA", name=f"sq1{eng}{n}", bufs=1)
                    sq2 = tr.tile([P, TB], f32r, tag="tmpB", name=f"sq2{eng}{n}", bufs=1)
                    nc.gpsimd.tensor_mul(sq1[:], x1[:], x1[:])
                    nc.gpsimd.tensor_mul(sq2[:], x2[:], x2[:])
                    ps_s = ps.tile([HPC, TB], f32, tag="psA", name=f"pss{eng}{n}")
                    nc.tensor.matmul(ps_s[:], lhsT=ind32_r[:], rhs=sq1[:],
                                     start=True, stop=False)
                    nc.tensor.matmul(ps_s[:], lhsT=ind32_r[:], rhs=sq2[:],
                                     start=False, stop=True)
                    invc = tr.tile([HPC, TB], f32r, tag="invc", name=f"invc{eng}{n}")
                    nc.scalar.activation(invc[:], ps_s[:], Ln,
                                         bias=eps_t[:], scale=1.0 / 64.0)
                    nc.scalar.activation(invc[:], invc[:], Exp, scale=-0.5)
                    ps_b = ps.tile([P, TB], f32, tag="psB", name=f"psb{eng}{n}")
                    nc.tensor.matmul(ps_b[:], lhsT=bc32_r[:], rhs=invc[:],
                                     start=True, stop=True)
                    nc.vector.tensor_mul(x1[:], x1[:], ps_b[:])
                    nc.vector.tensor_mul(x2[:], x2[:], ps_b[:])
                    # rope
                    m_a = tr.tile([P, TB], f32, tag="tmpA", name=f"ma{eng}{n}", bufs=1)
                    m_b = tr.tile([P, TB], f32, tag="tmpB", name=f"mb{eng}{n}", bufs=1)
                    rc1 = tr.tile([P, TB], f32r, tag="roch1", name=f"rc1{eng}{n}", bufs=1)
                    rc2 = tr.tile([P, TB], f32r, tag="roch2", name=f"rc2{eng}{n}", bufs=1)
                    e.tensor_mul(m_a[:], x1[:], cosr_t[:, nsl])
                    e.tensor_mul(m_b[:], x2[:], sinr_t[:, nsl])
                    e.tensor_add(rc1[:], m_a[:], m_b[:])
                    m_c = tr.tile([P, TB], f32, tag="tmpA", name=f"mc{eng}{n}", bufs=1)
                    m_d = tr.tile([P, TB], f32, tag="tmpB", name=f"md{eng}{n}", bufs=1)
                    e.tensor_mul(m_c[:], x2[:], cosr_t[:, nsl])
                    e.tensor_mul(m_d[:], x1[:], sinr_t[:, nsl])
                    e.tensor_sub(rc2[:], m_c[:], m_d[:])
                    # repack: head h rows 32h..32h+32 of (rc1|rc2) ->
                    # q: qT_r[h//2] rows 64*(h%2)..; k: kTe/kTo (zero-padded)
                    for h in range(HPC):
                        if eng == "q":
                            dst = dstT[h // 2]
                            rb = 64 * (h % 2)
                        else:
                            dst = (kTe if h % 2 == 0 else kTo)[h // 2]
                            rb = 64 * (h % 2)
                        hs = slice(32 * h, 32 * h + 32)
                        nc.sync.dma_start(out=dst[rb:rb + 32, nsl], in_=rc1[hs, :])
                        nc.sync.dma_start(out=dst[rb + 32:rb + 64, nsl], in_=rc2[hs, :])

            # ================= Phase 3: attention =================
            for h in range(HPC):
                cch = h // 2
                kT_h = (kTe if h % 2 == 0 else kTo)[cch]
                rsl = slice(64 * (h % 2), 64 * (h % 2) + 64)
                pa = "psC"
                ya = "psD"
                et_tag = "expT" if h % 2 == 0 else "expT2"
                for j in range(NB):
                    jsl = slice(j * TB, (j + 1) * TB)
                    n_k = 4 * j + 4
                    Yh = ps.tile([65, TB], f32, tag=ya, name=f"Y{h}_{j}")
                    for k in range(n_k):
                        ksl = slice(k * P, (k + 1) * P)
                        st, sp = (k == 0), (k == n_k - 1)
                        r = k - 4 * j          # >=0 on diagonal blocks
                        # cols t < 128r of this block are fully masked; trim
                        # matmuls to N>=256 (f32r full-rate) and exp always.
                        mtrim = 128 * r if 0 < r <= 2 else 0
                        etrim = 128 * r if r > 0 else 0
                        msl = slice(mtrim, TB)
                        esl = slice(etrim, TB)
                        S0 = ps.tile([P, TB], f32, tag=pa, name=f"S{h}_{j}_{k}")
                        nc.tensor.matmul(S0[:, msl], lhsT=kT_h[:, ksl],
                                         rhs=qT_r[cch][:, j * TB + mtrim:(j + 1) * TB],
                                         start=True, stop=True)
                        e0 = tr.tile([P, TB], f32r, tag=et_tag,
                                     name=f"e{h}_{j}_{k}", bufs=3)
                        nc.scalar.activation(e0[:, esl], S0[:, esl], Exp, scale=0.125)
                        if r >= 0:  # diagonal: apply causal mask (zero-fills left)
                            e0m = tr.tile([P, TB], f32r, tag=et_tag,
                                          name=f"em{h}_{j}_{k}", bufs=3)
                            nc.gpsimd.affine_select(
                                out=e0m[:], in_=e0[:], pattern=[[1, TB]],
                                compare_op=Alu.is_ge, fill=0.0,
                                base=-128 * r, channel_multiplier=-1)
                            e0 = e0m
                        nc.tensor.matmul(Yh[:, msl], lhsT=v_r[k][:, 65 * h:65 * h + 65],
                                         rhs=e0[:, msl], start=st, stop=sp)
                    # copy out: y rows + den row (SBUF bounce; DMA shifts rows)
                    yb = tr.tile([65, TB], f32r, tag="cpbuf", name=f"yb{h}_{j}",
                                 bufs=3, padded_shape=[P, TB])
                    nc.vector.tensor_copy(yb[:], Yh[:])
                    nc.sync.dma_start(out=yT_sb[cch][rsl, jsl], in_=yb[0:64, :])
                    nc.sync.dma_start(out=den_stack[32 * h:32 * h + 1, jsl],
                                      in_=yb[64:65, :])

            # ================= Phase 4: normalize + out-projection ======
            # invden = exp(-ln(den)) on rows 0,32,64,96 (others memset to 1)
            invden_r = sb.tile([P, T], f32r, tag="invden", name="invden_r")
            nc.scalar.activation(den_stack[:], den_stack[:], Ln)
            nc.scalar.activation(invden_r[:], den_stack[:], Exp, scale=-1.0)
            for c in range(2):
                for n in range(NB):
                    nsl = slice(n * TB, (n + 1) * TB)
                    ps_i = ps.tile([P, TB], f32, tag="psA", name=f"psi{c}{n}")
                    nc.tensor.matmul(ps_i[:], lhsT=selpair_r[:, c * P:(c + 1) * P],
                                     rhs=invden_r[:, nsl], start=True, stop=True)
                    nc.vector.tensor_mul(yT_sb[c][:, nsl], yT_sb[c][:, nsl], ps_i[:])
            for o in range(8):
                osl = slice(o * P, (o + 1) * P)
                for n in range(NB):
                    nsl = slice(n * TB, (n + 1) * TB)
                    po = ps.tile([P, TB], f32, tag="psB", name=f"po{o}_{n}")
                    nc.tensor.matmul(po[:], lhsT=wp_r[0][:, osl], rhs=yT_sb[0][:, nsl],
                                     start=True, stop=False)
                    nc.tensor.matmul(po[:], lhsT=wp_r[1][:, osl], rhs=yT_sb[1][:, nsl],
                                     start=False, stop=True)
                    ob = tr.tile([P, TB], f32, tag="cpbuf", name=f"ob{o}_{n}", bufs=3)
                    nc.vector.tensor_copy(ob[:], po[:])
                    nc.sync.dma_start(out=out_d[osl, nsl], in_=ob[:])

    nc.compile()
    return nc


def _get_module():
    if "nc" not in _CACHE:
        _CACHE["nc"] = _build_module()
        _CACHE["consts"] = _build_consts()
    return _CACHE["nc"], _CACHE["consts"]


def _round_f32r(a, bits=10):
    u = np.ascontiguousarray(a, dtype=np.float32).view(np.uint32).astype(np.uint64)
    u = (u + (1 << (bits - 1))) & ~np.uint64((1 << bits) - 1)
    return np.minimum(u, 0xFFFFFFFF).astype(np.uint32).view(np.float32)


def _core_inputs(x, w_q, w_k, w_v, w_proj, core):
    """Build the per-core input map (numpy, host-side sharding)."""
    b = core // 4
    g = core % 4
    heads = [4 * g + j for j in range(HPC)]

    xt = _round_f32r(np.ascontiguousarray(x[b].T))        # [C, T]

    perm = np.empty(256, dtype=np.int64)
    for m in range(128):
        perm[m] = 64 * heads[m // 32] + (m % 32)             # x1 half
        perm[128 + m] = 64 * heads[m // 32] + 32 + (m % 32)  # x2 half
    wq = _round_f32r(np.ascontiguousarray(w_q[perm, :].T))   # [C, 256]
    wk = _round_f32r(np.ascontiguousarray(w_k[perm, :].T))

    # v weights with zero columns at 65h+64 (device writes the ones there)
    wv_aug = np.zeros((C, VW), dtype=np.float32)
    for j in range(HPC):
        wv_aug[:, 65 * j:65 * j + 64] = w_v[64 * heads[j]:64 * heads[j] + 64, :].T
    wv = _round_f32r(wv_aug)

    vperm = np.empty(256, dtype=np.int64)
    for m in range(256):
        vperm[m] = 64 * heads[m // 64] + (m % 64)
    wp = _round_f32r(np.ascontiguousarray(w_proj[:, vperm].T))  # [256, C]

    zeros = np.zeros((64, T), dtype=np.float32)
    return dict(xt=xt, wq=wq, wk=wk, wv=wv, wp=wp, zeros=zeros)


def kernel(x, w_q, w_k, w_v, w_proj, _trace=False, _trace_cores=None):
    from concourse.bass_utils import run_bass_kernel_spmd

    nc, consts = _get_module()
    x = np.asarray(x, dtype=np.float32)
    in_maps = []
    for core in range(N_CORES):
        m = _core_inputs(np.asarray(x), np.asarray(w_q), np.asarray(w_k),
                         np.asarray(w_v), np.asarray(w_proj), core)
        m.update(consts)
        in_maps.append(m)

    res = run_bass_kernel_spmd(nc, in_maps, list(range(N_CORES)),
                               trace=_trace, trace_cores=_trace_cores)
    outs = [res.results[c]["outT"] for c in range(N_CORES)]
    out = np.empty((B, T, C), dtype=np.float32)
    for b in range(B):
        acc = outs[4 * b].astype(np.float32)
        for g in range(1, 4):
            acc = acc + outs[4 * b + g]
        out[b] = acc.T
    if _trace:
        kernel._last_exec_time_ns = res.exec_time_ns
        kernel._last_results = res
    return out



# revision 5
# speedup vs baseline: 1.2115x; 1.2115x over previous
"""Causal self-attention (RMSNorm-QK + RoPE) Trainium2 Bass kernel, v2.

Problem: B=2, T=2048, C=1024, H=16 heads, D=64.
Sharding: 8 cores = 2 (batch) x 4 (head groups of 4 heads). Host sums the
4 column-parallel out-proj partials per batch and transposes.

v2 design (vs baseline ~360us):
- bf16 everywhere except PSUM accum, den/invden math, and the final output.
- Fully fused single pass over 4 token blocks of 512: projections -> RMS ->
  RoPE -> attention(j=n) -> normalize+out-proj(n). No serial phases.
- Scores matmuls packed 2 heads/instruction via K=64 row-tiling
  (base_partition 0/64 auto tile_position).
- exp: one ACTIVATE per (pair, key-block) over a [128,2,512] PSUM pair tile.
- One manual ACT table load (natural_log_exp_and_others) serves the RMS
  Ln/Exp rsqrt and the attention Exp: no table thrashing.
- Causal masks: 4 precomputed bf16 [128,2,512] tiles, applied with DVE mult.
- Denominator via ones-column in v (M=65 AV); 1/den via DVE
  reciprocal_approx_fast on f32; bf16 den storage.
- Single big weight DMA + 3D-AP x loads to avoid startup DMA serialization.
"""

import sys

for _p in ("/opt/trn_rl_repo",):
    if _p not in sys.path:
        sys.path.append(_p)

import numpy as np

B, T, C = 2, 2048, 1024
H_TOT, D = 16, 64
HPC = 4               # heads per core
N_CORES = 8
P = 128
NB = 4                # token blocks
TB = 512              # token block size
KCH = 8               # C / 128 contraction chunks
RMS_EPS = 1.1920928955078125e-07
ROPE_BASE = 10000.0
ACT_SET_LN_EXP = 6    # natural_log_exp_and_others in act_info.json

_CACHE = {}


def _build_consts():
    import ml_dtypes
    bf = ml_dtypes.bfloat16
    inv_freq = (1.0 / (ROPE_BASE ** (np.arange(0, D, 2, dtype=np.float32) / np.float32(D)))).astype(np.float32)
    pos = np.arange(T, dtype=np.float32)
    freqs = np.outer(pos, inv_freq).astype(np.float32)      # [T, 32]
    cos = np.cos(freqs).astype(np.float32)
    sin = np.sin(freqs).astype(np.float32)
    cosr = np.ascontiguousarray(np.tile(cos.T, (HPC, 1))).astype(bf)   # [128, T]
    sinr = np.ascontiguousarray(np.tile(sin.T, (HPC, 1))).astype(bf)
    # ind32 [128, 4]: per-32-row-group summing matrix (lhsT for RMS sums)
    ind32 = np.zeros((P, HPC), dtype=np.float32)
    for p_ in range(P):
        ind32[p_, p_ // 32] = 1.0
    # bc32 [4, 128]: broadcast inv (4 heads) to 32-row groups (lhsT)
    bc32 = np.zeros((HPC, P), dtype=np.float32)
    for p_ in range(P):
        bc32[p_ // 32, p_] = 1.0
    # selpair [128, 256]: chunk c: col m -> den row 32*(2c + m//64)
    selpair = np.zeros((P, 2 * P), dtype=np.float32)
    for c in range(2):
        for m in range(P):
            selpair[32 * (2 * c + m // 64), 128 * c + m] = 1.0
    return dict(cosr=cosr, sinr=sinr, ind32=ind32.astype(bf),
                bc32=bc32.astype(bf), selpair=selpair.astype(bf))


def _build_module():
    import concourse.bacc as bacc
    import concourse.mybir as mybir
    import concourse.tile as tile

    f32 = mybir.dt.float32
    bf16 = mybir.dt.bfloat16
    Exp = mybir.ActivationFunctionType.Exp
    Ln = mybir.ActivationFunctionType.Ln
    Alu = mybir.AluOpType

    nc = bacc.Bacc("TRN2", target_bir_lowering=False, debug=False,
                   num_devices=N_CORES)

    # DRAM tensors. x is [128, 8, T] (chunk k at [:, k, :], row p = chan 128k+p)
    xt_d = nc.dram_tensor("xt", [P, KCH, T], bf16, kind="ExternalInput").ap()
    wq_d = nc.dram_tensor("wq", [P, KCH, 256], bf16, kind="ExternalInput").ap()
    wk_d = nc.dram_tensor("wk", [P, KCH, 256], bf16, kind="ExternalInput").ap()
    wv_d = nc.dram_tensor("wv", [P, KCH, 256], bf16, kind="ExternalInput").ap()
    wp_d = nc.dram_tensor("wp", [P, 2, C], bf16, kind="ExternalInput").ap()
    cosr_d = nc.dram_tensor("cosr", [P, T], bf16, kind="ExternalInput").ap()
    sinr_d = nc.dram_tensor("sinr", [P, T], bf16, kind="ExternalInput").ap()
    ind32_d = nc.dram_tensor("ind32", [P, HPC], bf16, kind="ExternalInput").ap()
    bc32_d = nc.dram_tensor("bc32", [HPC, P], bf16, kind="ExternalInput").ap()
    selpair_d = nc.dram_tensor("selpair", [P, 2 * P], bf16, kind="ExternalInput").ap()
    out_d = nc.dram_tensor("outT", [C, T], f32, kind="ExternalOutput").ap()

    with tile.TileContext(nc) as tc:
        nc.scalar.add_instruction(mybir.InstLoadActFuncSet(
            name=nc.get_next_instruction_name(),
            act_func_set_id=ACT_SET_LN_EXP, ins=[], outs=[]))

        with (
            tc.tile_pool(name="sb", bufs=1) as sb,
            tc.tile_pool(name="tr", bufs=2) as tr,
            tc.tile_pool(name="ps", bufs=1, space="PSUM") as ps,
        ):
            # ---- persistent SBUF ----
            def load(name, dram, shape, dt=bf16):
                t = sb.tile(shape, dt, tag=name, name=name)
                nc.sync.dma_start(out=t[:], in_=dram)
                return t

            # load order = need order: wq/wk first, wp last
            wq_t = load("wq", wq_d[:, :, :], [P, KCH, 256])
            wk_t = load("wk", wk_d[:, :, :], [P, KCH, 256])
            wv_t = load("wv", wv_d[:, :, :], [P, KCH, 256])
            ind32_t = load("ind32", ind32_d[:, :], [P, HPC])
            bc32_t = load("bc32", bc32_d[:, :], [HPC, P])
            cosr_t = load("cosr", cosr_d[:, :], [P, T])
            sinr_t = load("sinr", sinr_d[:, :], [P, T])
            selpair_t = load("selpair", selpair_d[:, :], [P, 2 * P])
            wp_t = load("wp", wp_d[:, :, :], [P, 2, C])

            qT = [sb.tile([P, T], bf16, tag=f"qT{c}", name=f"qT{c}")
                  for c in range(2)]
            kT = [sb.tile([P, T], bf16, tag=f"kT{c}", name=f"kT{c}")
                  for c in range(2)]
            v_r = [sb.tile([P, HPC, 65], bf16, tag=f"v{s}", name=f"v{s}")
                   for s in range(T // P)]
            for s in range(T // P):
                nc.gpsimd.memset(v_r[s][:, :, 64:65], 1.0)
            yT = [sb.tile([P, T], bf16, tag=f"yT{c}", name=f"yT{c}")
                  for c in range(2)]
            den_bf = sb.tile([P, T], bf16, tag="denbf", name="den_bf")
            nc.gpsimd.memset(den_bf[:], 1.0)

            # masks: mask_r [128, 2, 512] bf16, 1 where key p <= query q
            # within a diagonal block with key offset 128*r (q >= p + 128r).
            masks = []
            for r in range(4):
                m = sb.tile([P, 2, TB], bf16, tag=f"mask{r}", name=f"mask{r}")
                nc.gpsimd.memset(m[:], 1.0)
                # zero where q < p + 128 r  <=>  iota = -p + q - 128r < 0;
                # keep (copy in_) where iota >= 0. in 3D: [[0,2],[1,512]]
                nc.gpsimd.affine_select(
                    out=m[:], in_=m[:], pattern=[[0, 2], [1, TB]],
                    compare_op=Alu.is_ge, fill=0.0,
                    base=-P * r, channel_multiplier=-1)
                masks.append(m)

            eps_t = sb.tile([HPC, 1], f32, tag="epst", name="eps_t")
            nc.gpsimd.memset(eps_t[:], RMS_EPS)

            invden_f = [sb.tile([P, TB], f32, tag="invdf", name=f"invdf{n}",
                                bufs=2) for n in range(NB)]
            invden_b = [sb.tile([P, TB], bf16, tag="invdb", name=f"invdb{n}",
                                bufs=2) for n in range(NB)]

            def rope_one(eng, x_pair, invb, rc1, rc2, nsl, nm):
                """rc1 = (x1 cos + x2 sin) inv ; rc2 = (x2 cos - x1 sin) inv."""
                x1 = x_pair[:, 0, :]
                x2 = x_pair[:, 1, :]
                ma = tr.tile([P, TB], bf16, tag=f"ma{nm}", name=f"ma{nm}", bufs=2)
                mb = tr.tile([P, TB], bf16, tag=f"mb{nm}", name=f"mb{nm}", bufs=2)
                eng.tensor_mul(ma[:], x1, cosr_t[:, nsl])
                eng.tensor_mul(mb[:], x2, sinr_t[:, nsl])
                eng.tensor_add(ma[:], ma[:], mb[:])
                eng.tensor_mul(rc1[:], ma[:], invb[:])
                mc = tr.tile([P, TB], bf16, tag=f"mc{nm}", name=f"mc{nm}", bufs=2)
                md = tr.tile([P, TB], bf16, tag=f"md{nm}", name=f"md{nm}", bufs=2)
                eng.tensor_mul(mc[:], x2, cosr_t[:, nsl])
                eng.tensor_mul(md[:], x1, sinr_t[:, nsl])
                eng.tensor_sub(mc[:], mc[:], md[:])
                eng.tensor_mul(rc2[:], mc[:], invb[:])

            for n in range(NB):
                nsl = slice(n * TB, (n + 1) * TB)
                xr = tr.tile([P, KCH, TB], bf16, tag="xr", name=f"xr{n}", bufs=2)
                nc.sync.dma_start(out=xr[:], in_=xt_d[:, :, nsl])

                # ---- q/k projections + RMS inv + RoPE + repack ----
                for tens, w_t, dstT, eng_nm in (("q", wq_t, qT, "v"),
                                                ("k", wk_t, kT, "g")):
                    xp = tr.tile([P, 2, TB], bf16, tag="xp", name=f"xp{tens}{n}",
                                 bufs=2)
                    for half in range(2):
                        pg = ps.tile([P, TB], f32, tag="WK", name=f"p{tens}{half}_{n}",
                                     bufs=2)
                        for k in range(KCH):
                            nc.tensor.matmul(
                                pg[:], lhsT=w_t[:, k, 128 * half:128 * half + 128],
                                rhs=xr[:, k, :], start=(k == 0), stop=(k == KCH - 1))
                        nc.vector.tensor_copy(xp[:, half, :], pg[:])
                    # squares + per-head sums
                    sq = tr.tile([P, 2, TB], bf16, tag="sq", name=f"sq{tens}{n}",
                                 bufs=2)
                    nc.gpsimd.tensor_mul(sq[:], xp[:], xp[:])
                    msp = ps.tile([HPC, TB], f32, tag="WK", name=f"ms{tens}{n}",
                                  bufs=2)
                    nc.tensor.matmul(msp[:], lhsT=ind32_t[:], rhs=sq[:, 0, :],
                                     start=True, stop=False)
                    nc.tensor.matmul(msp[:], lhsT=ind32_t[:], rhs=sq[:, 1, :],
                                     start=False, stop=True)
                    invc = tr.tile([HPC, TB], bf16, tag="invc", name=f"invc{tens}{n}",
                                   bufs=2)
                    nc.scalar.activation(invc[:], msp[:], Ln,
                                         bias=eps_t[:], scale=1.0 / 64.0)
                    nc.scalar.activation(invc[:], invc[:], Exp, scale=-0.5)
                    invp = ps.tile([P, TB], f32, tag="WK", name=f"invp{tens}{n}",
                                   bufs=2)
                    nc.tensor.matmul(invp[:], lhsT=bc32_t[:], rhs=invc[:],
                                     start=True, stop=True)
                    invb = tr.tile([P, TB], bf16, tag="invb", name=f"invb{tens}{n}",
                                   bufs=2)
                    nc.vector.tensor_copy(invb[:], invp[:])
                    rc1 = tr.tile([P, TB], bf16, tag=f"rc1{tens}", name=f"rc1{tens}{n}",
                                  bufs=2)
                    rc2 = tr.tile([P, TB], bf16, tag=f"rc2{tens}", name=f"rc2{tens}{n}",
                                  bufs=2)
                    eng = nc.vector if eng_nm == "v" else nc.gpsimd
                    rope_one(eng, xp, invb, rc1, rc2, nsl, tens + str(n))
                    # repack into qT/kT: head h dims = rows 32h of rc1|rc2
                    for c in range(2):
                        dst = dstT[c]
                        for h2, src in ((0, rc1), (1, rc2)):
                            nc.sync.dma_start(
                                out=dst[32 * h2:32 * h2 + 32, nsl],
                                in_=src[64 * c:64 * c + 32, :])
                            nc.sync.dma_start(
                                out=dst[64 + 32 * h2:64 + 32 * h2 + 32, nsl],
                                in_=src[64 * c + 32:64 * c + 64, :])

                # ---- v projection ----
                for s_rel in range(4):
                    pv = ps.tile([P, HPC, 64], f32, tag="WK", name=f"pv{n}_{s_rel}",
                                 bufs=2)
                    for k in range(KCH):
                        nc.tensor.matmul(
                            pv[:], lhsT=xr[:, k, s_rel * P:(s_rel + 1) * P],
                            rhs=wv_t[:, k, :], start=(k == 0), stop=(k == KCH - 1))
                    nc.vector.tensor_copy(v_r[4 * n + s_rel][:, :, 0:64], pv[:])

                # ---- attention j = n ----
                for c in range(2):
                    Y = ps.tile([65, 2, TB], f32, tag="YP", name=f"Y{c}_{n}",
                                bufs=1)
                    n_k = 4 * n + 4
                    for k in range(n_k):
                        r = k - 4 * n
                        mt = P * r if r > 0 else 0
                        ksl = slice(k * P, (k + 1) * P)
                        qsl = slice(n * TB + mt, (n + 1) * TB)
                        S = ps.tile([P, 2, TB], f32, tag="SP", name=f"S{c}{n}_{k}",
                                    bufs=2)
                        nc.tensor.matmul(S[:, 0, mt:TB], lhsT=kT[c][0:64, ksl],
                                         rhs=qT[c][0:64, qsl], start=True, stop=True)
                        nc.tensor.matmul(S[:, 1, mt:TB], lhsT=kT[c][64:128, ksl],
                                         rhs=qT[c][64:128, qsl], start=True, stop=True)
                        e0 = tr.tile([P, 2, TB], bf16, tag="e0", name=f"e{c}{n}_{k}",
                                     bufs=3)
                        nc.scalar.activation(e0[:, :, mt:TB], S[:, :, mt:TB],
                                             Exp, scale=0.125)
                        if r >= 0:
                            em = tr.tile([P, 2, TB], bf16, tag="em",
                                         name=f"em{c}{n}_{k}", bufs=2)
                            nc.vector.tensor_mul(em[:, :, mt:TB], e0[:, :, mt:TB],
                                                 masks[r][:, :, mt:TB])
                            e0 = em
                        for h2 in range(2):
                            nc.tensor.matmul(
                                Y[:, h2, mt:TB],
                                lhsT=v_r[k][:, 2 * c + h2, :],
                                rhs=e0[:, h2, mt:TB],
                                start=(k == 0), stop=(k == n_k - 1))
                    yb = tr.tile([65, 2, TB], bf16, tag="yb", name=f"yb{c}{n}",
                                 bufs=2, padded_shape=[P, 2, TB])
                    nc.vector.tensor_copy(yb[:], Y[:])
                    for h2 in range(2):
                        nc.sync.dma_start(out=yT[c][64 * h2:64 * h2 + 64, nsl],
                                          in_=yb[0:64, h2, :])
                        nc.sync.dma_start(
                            out=den_bf[32 * (2 * c + h2):32 * (2 * c + h2) + 1, nsl],
                            in_=yb[64:65, h2, :])

                # ---- normalize + out-projection for block n ----
                den_f = tr.tile([P, TB], f32, tag="denf", name=f"denf{n}", bufs=2)
                nc.vector.tensor_copy(den_f[:], den_bf[:, nsl])
                nc.vector.reciprocal_approx_fast(out=invden_f[n][:], in_=den_f[:])
                nc.vector.tensor_copy(invden_b[n][:], invden_f[n][:])
                for c in range(2):
                    psi = ps.tile([P, TB], f32, tag="WK", name=f"psi{c}{n}",
                                  bufs=2)
                    nc.tensor.matmul(psi[:], lhsT=selpair_t[:, c * P:(c + 1) * P],
                                     rhs=invden_b[n][:], start=True, stop=True)
                    nc.vector.tensor_mul(yT[c][:, nsl], yT[c][:, nsl], psi[:])
                for o in range(8):
                    osl = slice(o * P, (o + 1) * P)
                    po = ps.tile([P, TB], f32, tag="WK", name=f"po{o}_{n}",
                                 bufs=2)
                    nc.tensor.matmul(po[:], lhsT=wp_t[:, 0, osl], rhs=yT[0][:, nsl],
                                     start=True, stop=False)
                    nc.tensor.matmul(po[:], lhsT=wp_t[:, 1, osl], rhs=yT[1][:, nsl],
                                     start=False, stop=True)
                    ob = tr.tile([P, TB], f32, tag="ob", name=f"ob{o}_{n}", bufs=3)
                    if o % 2 == 0:
                        nc.vector.tensor_copy(ob[:], po[:])
                    else:
                        nc.scalar.copy(ob[:], po[:])
                    nc.sync.dma_start(out=out_d[osl, nsl], in_=ob[:])

    nc.compile()
    return nc


def _get_module():
    if "nc" not in _CACHE:
        _CACHE["nc"] = _build_module()
        _CACHE["consts"] = _build_consts()
    return _CACHE["nc"], _CACHE["consts"]


def _core_inputs(x, w_q, w_k, w_v, w_proj, core):
    import ml_dtypes
    bf = ml_dtypes.bfloat16
    b = core // 4
    g = core % 4
    heads = [4 * g + j for j in range(HPC)]

    xt = np.ascontiguousarray(x[b].T).reshape(KCH, P, T).transpose(1, 0, 2)
    xt = np.ascontiguousarray(xt).astype(bf)                # [128, 8, T]

    def chunked(a):
        # [C, F] -> [128, C//128, F] with chunk k = rows 128k..128k+127
        F = a.shape[1]
        return np.ascontiguousarray(
            a.reshape(a.shape[0] // P, P, F).transpose(1, 0, 2)).astype(bf)

    perm = np.empty(256, dtype=np.int64)
    for m in range(128):
        perm[m] = 64 * heads[m // 32] + (m % 32)             # x1 half
        perm[128 + m] = 64 * heads[m // 32] + 32 + (m % 32)  # x2 half
    wq = chunked(np.ascontiguousarray(w_q[perm, :].T))       # [128, 8, 256]
    wk = chunked(np.ascontiguousarray(w_k[perm, :].T))

    vperm = np.empty(256, dtype=np.int64)
    for m in range(256):
        vperm[m] = 64 * heads[m // 64] + (m % 64)
    wv = chunked(np.ascontiguousarray(w_v[vperm, :].T))      # [128, 8, 256]
    wp = chunked(np.ascontiguousarray(w_proj[:, vperm].T))   # [128, 2, C]
    return dict(xt=xt, wq=wq, wk=wk, wv=wv, wp=wp)


def kernel(x, w_q, w_k, w_v, w_proj, _trace=False, _trace_cores=None):
    from concourse.bass_utils import run_bass_kernel_spmd

    nc, consts = _get_module()
    x = np.asarray(x, dtype=np.float32)
    in_maps = []
    for core in range(N_CORES):
        m = _core_inputs(np.asarray(x), np.asarray(w_q), np.asarray(w_k),
                         np.asarray(w_v), np.asarray(w_proj), core)
        m.update(consts)
        in_maps.append(m)

    res = run_bass_kernel_spmd(nc, in_maps, list(range(N_CORES)),
                               trace=_trace, trace_cores=_trace_cores)
    outs = [res.results[c]["outT"] for c in range(N_CORES)]
    out = np.empty((B, T, C), dtype=np.float32)
    for b in range(B):
        acc = outs[4 * b].astype(np.float32)
        for g in range(1, 4):
            acc = acc + outs[4 * b + g]
        out[b] = acc.T
    if _trace:
        kernel._last_exec_time_ns = res.exec_time_ns
        kernel._last_results = res
    return out


# revision 10
# speedup vs baseline: 1.4537x; 1.1999x over previous
"""Causal self-attention (RMSNorm-QK + RoPE) Trainium2 Bass kernel, v2.

Problem: B=2, T=2048, C=1024, H=16 heads, D=64.
Sharding: 8 cores = 2 (batch) x 4 (head groups of 4 heads). Host sums the
4 column-parallel out-proj partials per batch and transposes.

v2 design (vs baseline ~360us):
- bf16 everywhere except PSUM accum, den/invden math, and the final output.
- Fully fused single pass over 4 token blocks of 512: projections -> RMS ->
  RoPE -> attention(j=n) -> normalize+out-proj(n). No serial phases.
- Scores matmuls packed 2 heads/instruction via K=64 row-tiling
  (base_partition 0/64 auto tile_position).
- exp: one ACTIVATE per (pair, key-block) over a [128,2,512] PSUM pair tile.
- One manual ACT table load (natural_log_exp_and_others) serves the RMS
  Ln/Exp rsqrt and the attention Exp: no table thrashing.
- Causal masks: 4 precomputed bf16 [128,2,512] tiles, applied with DVE mult.
- Denominator via ones-column in v (M=65 AV); 1/den via DVE
  reciprocal_approx_fast on f32; bf16 den storage.
- Single big weight DMA + 3D-AP x loads to avoid startup DMA serialization.
"""

import sys

for _p in ("/opt/trn_rl_repo",):
    if _p not in sys.path:
        sys.path.append(_p)

import numpy as np

B, T, C = 2, 2048, 1024
H_TOT, D = 16, 64
HPC = 4               # heads per core
N_CORES = 8
P = 128
NB = 4                # token blocks
TB = 512              # token block size
KCH = 8               # C / 128 contraction chunks
RMS_EPS = 1.1920928955078125e-07
ROPE_BASE = 10000.0
ACT_SET_LN_EXP = 6    # natural_log_exp_and_others in act_info.json

_CACHE = {}


def _build_consts():
    import ml_dtypes
    bf = ml_dtypes.bfloat16
    inv_freq = (1.0 / (ROPE_BASE ** (np.arange(0, D, 2, dtype=np.float32) / np.float32(D)))).astype(np.float32)
    pos = np.arange(T, dtype=np.float32)
    freqs = np.outer(pos, inv_freq).astype(np.float32)      # [T, 32]
    cos = np.cos(freqs).astype(np.float32)
    sin = np.sin(freqs).astype(np.float32)
    cosr = np.ascontiguousarray(np.tile(cos.T, (HPC, 1))).astype(bf)   # [128, T]
    sinr = np.ascontiguousarray(np.tile(sin.T, (HPC, 1))).astype(bf)
    # ind32 [128, 4]: per-32-row-group summing matrix (lhsT for RMS sums)
    ind32 = np.zeros((P, HPC), dtype=np.float32)
    for p_ in range(P):
        ind32[p_, p_ // 32] = 1.0
    # bc32 [4, 128]: broadcast inv (4 heads) to 32-row groups (lhsT)
    bc32 = np.zeros((HPC, P), dtype=np.float32)
    for p_ in range(P):
        bc32[p_ // 32, p_] = 1.0
    # selpair [128, 256]: chunk c: col m -> den row 32*(2c + m//64)
    selpair = np.zeros((P, 2 * P), dtype=np.float32)
    for c in range(2):
        for m in range(P):
            selpair[32 * (2 * c + m // 64), 128 * c + m] = 1.0
    return dict(cosr=cosr, sinr=sinr, ind32=ind32.astype(bf),
                bc32=bc32.astype(bf), selpair=selpair.astype(bf))


def _build_module():
    import concourse.bacc as bacc
    import concourse.mybir as mybir
    import concourse.tile as tile

    f32 = mybir.dt.float32
    bf16 = mybir.dt.bfloat16
    Exp = mybir.ActivationFunctionType.Exp
    Ln = mybir.ActivationFunctionType.Ln
    Alu = mybir.AluOpType

    nc = bacc.Bacc("TRN2", target_bir_lowering=False, debug=False,
                   num_devices=N_CORES)

    # DRAM tensors. x is [128, 8, T] (chunk k at [:, k, :], row p = chan 128k+p)
    xt_d = nc.dram_tensor("xt", [P, KCH, T], bf16, kind="ExternalInput").ap()
    wq_d = nc.dram_tensor("wq", [P, KCH, 256], bf16, kind="ExternalInput").ap()
    wk_d = nc.dram_tensor("wk", [P, KCH, 256], bf16, kind="ExternalInput").ap()
    wv_d = nc.dram_tensor("wv", [P, KCH, 256], bf16, kind="ExternalInput").ap()
    wp_d = nc.dram_tensor("wp", [P, 2, C], bf16, kind="ExternalInput").ap()
    cosr_d = nc.dram_tensor("cosr", [P, T], bf16, kind="ExternalInput").ap()
    sinr_d = nc.dram_tensor("sinr", [P, T], bf16, kind="ExternalInput").ap()
    ind32_d = nc.dram_tensor("ind32", [P, HPC], bf16, kind="ExternalInput").ap()
    bc32_d = nc.dram_tensor("bc32", [HPC, P], bf16, kind="ExternalInput").ap()
    selpair_d = nc.dram_tensor("selpair", [P, 2 * P], bf16, kind="ExternalInput").ap()
    out_d = nc.dram_tensor("outT", [C, T], f32, kind="ExternalOutput").ap()

    with tile.TileContext(nc) as tc:
        nc.scalar.add_instruction(mybir.InstLoadActFuncSet(
            name=nc.get_next_instruction_name(),
            act_func_set_id=ACT_SET_LN_EXP, ins=[], outs=[]))

        with (
            tc.tile_pool(name="sb", bufs=1) as sb,
            tc.tile_pool(name="tr", bufs=2) as tr,
            tc.tile_pool(name="ps", bufs=1, space="PSUM") as ps,
        ):
            # ---- persistent SBUF ----
            def load(name, dram, shape, dt=bf16):
                t = sb.tile(shape, dt, tag=name, name=name)
                nc.sync.dma_start(out=t[:], in_=dram)
                return t

            # load order = need order: wq/wk first, wp last
            wq_t = load("wq", wq_d[:, :, :], [P, KCH, 256])
            wk_t = load("wk", wk_d[:, :, :], [P, KCH, 256])
            wv_t = load("wv", wv_d[:, :, :], [P, KCH, 256])
            ind32_t = load("ind32", ind32_d[:, :], [P, HPC])
            bc32_t = load("bc32", bc32_d[:, :], [HPC, P])
            cosr_t = load("cosr", cosr_d[:, :], [P, T])
            sinr_t = load("sinr", sinr_d[:, :], [P, T])
            selpair_t = load("selpair", selpair_d[:, :], [P, 2 * P])
            wp_t = load("wp", wp_d[:, :, :], [P, 2, C])

            qT = [sb.tile([P, T], bf16, tag=f"qT{c}", name=f"qT{c}")
                  for c in range(2)]
            kT = [sb.tile([P, T], bf16, tag=f"kT{c}", name=f"kT{c}")
                  for c in range(2)]
            v_r = [sb.tile([P, HPC, 65], bf16, tag=f"v{s}", name=f"v{s}")
                   for s in range(T // P)]
            for s in range(T // P):
                nc.gpsimd.memset(v_r[s][:, :, 64:65], 1.0)
            yT = [sb.tile([P, T], bf16, tag=f"yT{c}", name=f"yT{c}")
                  for c in range(2)]
            den_bf = sb.tile([P, T], bf16, tag="denbf", name="den_bf")
            nc.gpsimd.memset(den_bf[:], 1.0)

            # masks: mask_r [128, 2, 512] bf16, 1 where key p <= query q
            # within a diagonal block with key offset 128*r (q >= p + 128r).
            masks = []
            for r in range(4):
                m = sb.tile([P, 2, TB], bf16, tag=f"mask{r}", name=f"mask{r}")
                nc.gpsimd.memset(m[:], 1.0)
                # zero where q < p + 128 r  <=>  iota = -p + q - 128r < 0;
                # keep (copy in_) where iota >= 0. in 3D: [[0,2],[1,512]]
                nc.gpsimd.affine_select(
                    out=m[:], in_=m[:], pattern=[[0, 2], [1, TB]],
                    compare_op=Alu.is_ge, fill=0.0,
                    base=-P * r, channel_multiplier=-1)
                masks.append(m)

            eps_t = sb.tile([HPC, 1], f32, tag="epst", name="eps_t")
            nc.gpsimd.memset(eps_t[:], RMS_EPS)

            invden_f = [sb.tile([P, TB], f32, tag="invdf", name=f"invdf{n}",
                                bufs=2) for n in range(NB)]
            invden_b = [sb.tile([P, TB], bf16, tag="invdb", name=f"invdb{n}",
                                bufs=2) for n in range(NB)]

            def rope_one(eng, x_pair, invb, rc1, rc2, nsl, nm):
                """rc1 = (x1 cos + x2 sin) inv ; rc2 = (x2 cos - x1 sin) inv."""
                x1 = x_pair[:, 0, :]
                x2 = x_pair[:, 1, :]
                ma = tr.tile([P, TB], bf16, tag=f"ma{nm}", name=f"ma{nm}", bufs=2)
                mb = tr.tile([P, TB], bf16, tag=f"mb{nm}", name=f"mb{nm}", bufs=2)
                eng.tensor_mul(ma[:], x1, cosr_t[:, nsl])
                eng.tensor_mul(mb[:], x2, sinr_t[:, nsl])
                eng.tensor_add(ma[:], ma[:], mb[:])
                eng.tensor_mul(rc1[:], ma[:], invb[:])
                mc = tr.tile([P, TB], bf16, tag=f"mc{nm}", name=f"mc{nm}", bufs=2)
                md = tr.tile([P, TB], bf16, tag=f"md{nm}", name=f"md{nm}", bufs=2)
                eng.tensor_mul(mc[:], x2, cosr_t[:, nsl])
                eng.tensor_mul(md[:], x1, sinr_t[:, nsl])
                eng.tensor_sub(mc[:], mc[:], md[:])
                eng.tensor_mul(rc2[:], mc[:], invb[:])

            def p1_block(n):
                """Projections + RMS inv + RoPE + repack + v for block n."""
                nsl = slice(n * TB, (n + 1) * TB)
                xr = tr.tile([P, KCH, TB], bf16, tag="xr", name=f"xr{n}", bufs=2)
                nc.sync.dma_start(out=xr[:], in_=xt_d[:, :, nsl])

                # ---- q/k projections + RMS inv + RoPE + repack ----
                for tens, w_t, dstT, eng_nm in (("q", wq_t, qT, "v"),
                                                ("k", wk_t, kT, "g")):
                    xp = tr.tile([P, 2, TB], bf16, tag="xp", name=f"xp{tens}{n}",
                                 bufs=2)
                    for half in range(2):
                        pg = ps.tile([P, TB], f32, tag="WK", name=f"p{tens}{half}_{n}",
                                     bufs=2)
                        for k in range(KCH):
                            nc.tensor.matmul(
                                pg[:], lhsT=w_t[:, k, 128 * half:128 * half + 128],
                                rhs=xr[:, k, :], start=(k == 0), stop=(k == KCH - 1))
                        nc.vector.tensor_copy(xp[:, half, :], pg[:])
                    # squares + per-head sums
                    sq = tr.tile([P, 2, TB], bf16, tag="sq", name=f"sq{tens}{n}",
                                 bufs=2)
                    nc.gpsimd.tensor_mul(sq[:], xp[:], xp[:])
                    msp = ps.tile([HPC, TB], f32, tag="WK", name=f"ms{tens}{n}",
                                  bufs=2)
                    nc.tensor.matmul(msp[:], lhsT=ind32_t[:], rhs=sq[:, 0, :],
                                     start=True, stop=False)
                    nc.tensor.matmul(msp[:], lhsT=ind32_t[:], rhs=sq[:, 1, :],
                                     start=False, stop=True)
                    invc = tr.tile([HPC, TB], bf16, tag="invc", name=f"invc{tens}{n}",
                                   bufs=2)
                    nc.scalar.activation(invc[:], msp[:], Ln,
                                         bias=eps_t[:], scale=1.0 / 64.0)
                    nc.scalar.activation(invc[:], invc[:], Exp, scale=-0.5)
                    invp = ps.tile([P, TB], f32, tag="WK", name=f"invp{tens}{n}",
                                   bufs=2)
                    nc.tensor.matmul(invp[:], lhsT=bc32_t[:], rhs=invc[:],
                                     start=True, stop=True)
                    invb = tr.tile([P, TB], bf16, tag="invb", name=f"invb{tens}{n}",
                                   bufs=2)
                    nc.vector.tensor_copy(invb[:], invp[:])
                    rc1 = tr.tile([P, TB], bf16, tag=f"rc1{tens}", name=f"rc1{tens}{n}",
                                  bufs=2)
                    rc2 = tr.tile([P, TB], bf16, tag=f"rc2{tens}", name=f"rc2{tens}{n}",
                                  bufs=2)
                    eng = nc.vector if eng_nm == "v" else nc.gpsimd
                    rope_one(eng, xp, invb, rc1, rc2, nsl, tens + str(n))
                    # repack into qT/kT: head h dims = rows 32h of rc1|rc2.
                    # Issue from gpsimd queue (q) / sync (k) to keep the
                    # critical repack off the congested sync stream.
                    dma_eng = nc.gpsimd if tens == "q" else nc.sync
                    for c in range(2):
                        dst = dstT[c]
                        for h2, src in ((0, rc1), (1, rc2)):
                            dma_eng.dma_start(
                                out=dst[32 * h2:32 * h2 + 32, nsl],
                                in_=src[64 * c:64 * c + 32, :])
                            dma_eng.dma_start(
                                out=dst[64 + 32 * h2:64 + 32 * h2 + 32, nsl],
                                in_=src[64 * c + 32:64 * c + 64, :])

                # ---- v projection ----
                for s_rel in range(4):
                    pv = ps.tile([P, HPC, 64], f32, tag="WK", name=f"pv{n}_{s_rel}",
                                 bufs=2)
                    for k in range(KCH):
                        nc.tensor.matmul(
                            pv[:], lhsT=xr[:, k, s_rel * P:(s_rel + 1) * P],
                            rhs=wv_t[:, k, :], start=(k == 0), stop=(k == KCH - 1))
                    nc.vector.tensor_copy(v_r[4 * n + s_rel][:, :, 0:64], pv[:])

            def attn_block(n):
                nsl = slice(n * TB, (n + 1) * TB)
                for c in range(2):
                    Y = ps.tile([65, 2, TB], f32, tag="YP", name=f"Y{c}_{n}",
                                bufs=1)
                    n_k = 4 * n + 4
                    for k in range(n_k):
                        r = k - 4 * n
                        mt = P * r if r > 0 else 0
                        ksl = slice(k * P, (k + 1) * P)
                        qsl = slice(n * TB + mt, (n + 1) * TB)
                        S = ps.tile([P, 2, TB], f32, tag="SP", name=f"S{c}{n}_{k}",
                                    bufs=2)
                        nc.tensor.matmul(S[:, 0, mt:TB], lhsT=kT[c][0:64, ksl],
                                         rhs=qT[c][0:64, qsl], start=True, stop=True)
                        nc.tensor.matmul(S[:, 1, mt:TB], lhsT=kT[c][64:128, ksl],
                                         rhs=qT[c][64:128, qsl], start=True, stop=True)
                        e0 = tr.tile([P, 2, TB], bf16, tag="e0", name=f"e{c}{n}_{k}",
                                     bufs=3)
                        nc.scalar.activation(e0[:, :, mt:TB], S[:, :, mt:TB],
                                             Exp, scale=0.125)
                        if r >= 0:
                            em = tr.tile([P, 2, TB], bf16, tag="em",
                                         name=f"em{c}{n}_{k}", bufs=2)
                            nc.vector.tensor_mul(em[:, :, mt:TB], e0[:, :, mt:TB],
                                                 masks[r][:, :, mt:TB])
                            e0 = em
                        for h2 in range(2):
                            nc.tensor.matmul(
                                Y[:, h2, mt:TB],
                                lhsT=v_r[k][:, 2 * c + h2, :],
                                rhs=e0[:, h2, mt:TB],
                                start=(k == 0), stop=(k == n_k - 1))
                    yb = tr.tile([65, 2, TB], bf16, tag="yb", name=f"yb{c}{n}",
                                 bufs=2, padded_shape=[P, 2, TB])
                    nc.vector.tensor_copy(yb[:], Y[:])
                    for h2 in range(2):
                        nc.sync.dma_start(out=yT[c][64 * h2:64 * h2 + 64, nsl],
                                          in_=yb[0:64, h2, :])
                        nc.sync.dma_start(
                            out=den_bf[32 * (2 * c + h2):32 * (2 * c + h2) + 1, nsl],
                            in_=yb[64:65, h2, :])

            def outproj_block(n):
                nsl = slice(n * TB, (n + 1) * TB)
                den_f = tr.tile([P, TB], f32, tag="denf", name=f"denf{n}", bufs=2)
                nc.vector.tensor_copy(den_f[:], den_bf[:, nsl])
                nc.vector.reciprocal_approx_fast(out=invden_f[n][:], in_=den_f[:])
                nc.vector.tensor_copy(invden_b[n][:], invden_f[n][:])
                for c in range(2):
                    psi = ps.tile([P, TB], f32, tag="WK", name=f"psi{c}{n}",
                                  bufs=2)
                    nc.tensor.matmul(psi[:], lhsT=selpair_t[:, c * P:(c + 1) * P],
                                     rhs=invden_b[n][:], start=True, stop=True)
                    nc.vector.tensor_mul(yT[c][:, nsl], yT[c][:, nsl], psi[:])
                for o in range(8):
                    osl = slice(o * P, (o + 1) * P)
                    po = ps.tile([P, TB], f32, tag="WK", name=f"po{o}_{n}",
                                 bufs=2)
                    nc.tensor.matmul(po[:], lhsT=wp_t[:, 0, osl], rhs=yT[0][:, nsl],
                                     start=True, stop=False)
                    nc.tensor.matmul(po[:], lhsT=wp_t[:, 1, osl], rhs=yT[1][:, nsl],
                                     start=False, stop=True)
                    ob = tr.tile([P, TB], f32, tag="ob", name=f"ob{o}_{n}", bufs=3)
                    nc.vector.tensor_copy(ob[:], po[:])
                    nc.sync.dma_start(out=out_d[osl, nsl], in_=ob[:])

            # Software pipeline: emit P1(n+1) before attention(n) so the
            # tensor stream has ready projection work while block n's
            # repack DMAs land.
            p1_block(0)
            for n in range(NB):
                if n + 1 < NB:
                    p1_block(n + 1)
                attn_block(n)
                outproj_block(n)

    nc.compile()
    return nc


def _get_module():
    if "nc" not in _CACHE:
        _CACHE["nc"] = _build_module()
        _CACHE["consts"] = _build_consts()
    return _CACHE["nc"], _CACHE["consts"]


def _core_inputs(x, w_q, w_k, w_v, w_proj, core):
    import ml_dtypes
    bf = ml_dtypes.bfloat16
    b = core // 4
    g = core % 4
    heads = [4 * g + j for j in range(HPC)]

    xt = np.ascontiguousarray(x[b].T).reshape(KCH, P, T).transpose(1, 0, 2)
    xt = np.ascontiguousarray(xt).astype(bf)                # [128, 8, T]

    def chunked(a):
        # [C, F] -> [128, C//128, F] with chunk k = rows 128k..128k+127
        F = a.shape[1]
        return np.ascontiguousarray(
            a.reshape(a.shape[0] // P, P, F).transpose(1, 0, 2)).astype(bf)

    perm = np.empty(256, dtype=np.int64)
    for m in range(128):
        perm[m] = 64 * heads[m // 32] + (m % 32)             # x1 half
        perm[128 + m] = 64 * heads[m // 32] + 32 + (m % 32)  # x2 half
    wq = chunked(np.ascontiguousarray(w_q[perm, :].T))       # [128, 8, 256]
    wk = chunked(np.ascontiguousarray(w_k[perm, :].T))

    vperm = np.empty(256, dtype=np.int64)
    for m in range(256):
        vperm[m] = 64 * heads[m // 64] + (m % 64)
    wv = chunked(np.ascontiguousarray(w_v[vperm, :].T))      # [128, 8, 256]
    wp = chunked(np.ascontiguousarray(w_proj[:, vperm].T))   # [128, 2, C]
    return dict(xt=xt, wq=wq, wk=wk, wv=wv, wp=wp)


def kernel(x, w_q, w_k, w_v, w_proj, _trace=False, _trace_cores=None):
    from concourse.bass_utils import run_bass_kernel_spmd

    nc, consts = _get_module()
    x = np.asarray(x, dtype=np.float32)
    in_maps = []
    for core in range(N_CORES):
        m = _core_inputs(np.asarray(x), np.asarray(w_q), np.asarray(w_k),
                         np.asarray(w_v), np.asarray(w_proj), core)
        m.update(consts)
        in_maps.append(m)

    res = run_bass_kernel_spmd(nc, in_maps, list(range(N_CORES)),
                               trace=_trace, trace_cores=_trace_cores)
    outs = [res.results[c]["outT"] for c in range(N_CORES)]
    out = np.empty((B, T, C), dtype=np.float32)
    for b in range(B):
        acc = outs[4 * b].astype(np.float32)
        for g in range(1, 4):
            acc = acc + outs[4 * b + g]
        out[b] = acc.T
    if _trace:
        kernel._last_exec_time_ns = res.exec_time_ns
        kernel._last_results = res
    return out
